# revision 22
# baseline (speedup 1.0000x reference)
"""Trainium2 Bass kernel for nn_AMXReversibleLayer.

Reference computation (RevNet-style additive coupling):
    x1, x2 = split(x, 2, axis=-1)      # x: [B, S, 2D] f32, each [B, S, D]
    y = concat([x1, x2 + x1 @ W], -1)  # W: [D, D] f32

Strategy: pure data-parallel. x [8, 32768, 256] is sharded along batch —
one batch element (32768 tokens) per NeuronCore, W replicated. No
collectives. The kernel is memory-bound: per core 32 MB in + 32 MB out.

Per-core kernel (Tile framework):
  - Tokens live on SBUF partitions (128/tile), the 256 features on the
    free axis, so DMAs move 1 KB-contiguous runs per token.
  - TensorE needs the contraction dim (d) on partitions, so each
    128-token x1 tile is transposed on the PE array (matmul vs identity)
    into PSUM, copied back to SBUF, then matmul'd against W.
  - h is added in-place into the x2 columns of the input tile and the
    whole tile goes back out with one DMA.

Constraint that shapes the engine assignment: an f32 matmul lowers to
LDWEIGHTS+MATMULT and the LW slot accepts only ONE sync-wait command
(walrus "Too many sync wait commands" otherwise). So every PE
instruction must depend on a single semaphore: ALL elementwise work
(const prep, x1 staging, PSUM->SBUF copies, adds) runs on the
VectorEngine, and PE never reads DMA'd data directly (x1 is staged
through a DVE copy first).
"""

import numpy as np

import concourse.bass as bass
import concourse.mybir as mybir
from concourse.bass_utils import run_bass_kernel_spmd
from concourse.masks import make_identity
from concourse.tile import TileContext

N_CORES = 8
B, S, TWO_D = 8, 32768, 256
D = 128
P = 128

TOKENS = (B * S) // N_CORES          # tokens per core = 32768
TILES = TOKENS // P                  # 256 tiles of 128 tokens
TILES_PER_GROUP = 32                 # 32 tiles -> 4 MB in-DMA, 32 KB runs
NGROUPS = TILES // TILES_PER_GROUP   # 8
BUNDLE = 4                           # tiles per PSUM bank ([128, 512] f32)

_CACHE = {}


def _build_nc() -> bass.Bass:
    nc = bass.Bass()
    x = nc.dram_tensor("x", [TOKENS, TWO_D], mybir.dt.float32, kind="ExternalInput")
    w = nc.dram_tensor("weight", [D, D], mybir.dt.float32, kind="ExternalInput")
    out = nc.dram_tensor("out", [TOKENS, TWO_D], mybir.dt.float32, kind="ExternalOutput")

    # [g, p, t, d] views: token = p*(NGROUPS*T) + g*T + t. Partition p
    # owns a CONTIGUOUS run of tokens, so each per-partition DMA run is
    # T*2D*4 = 16 KB contiguous (vs 1 KB with interleaved mapping) —
    # far fewer descriptors at full line rate. Compute doesn't care
    # which 128 tokens form a tile.
    xg = x.rearrange("(p g t) d -> g p t d", p=P, g=NGROUPS)
    og = out.rearrange("(p g t) d -> g p t d", p=P, g=NGROUPS)

    with TileContext(nc) as tc:
        with (
            tc.tile_pool(name="const", bufs=1) as const_pool,
            tc.tile_pool(name="io", bufs=3) as io_pool,
            tc.tile_pool(name="xT", bufs=4) as xT_pool,
            tc.tile_pool(name="psT", bufs=4, space="PSUM") as psT_pool,
            tc.tile_pool(name="psH", bufs=4, space="PSUM") as psH_pool,
        ):
            # Kick off the first big input DMA before anything else so
            # the memory pipe starts streaming immediately.
            xt0 = io_pool.tile([P, TILES_PER_GROUP * TWO_D], mybir.dt.float32, tag="xt")
            xt0_3 = xt0[:].rearrange("p (t d) -> p t d", d=TWO_D)
            nc.sync.dma_start(out=xt0_3, in_=xg[0])

            ident_raw = const_pool.tile([P, P], mybir.dt.float32)
            make_identity(nc, ident_raw[:])
            ident = const_pool.tile([P, P], mybir.dt.float32)
            nc.vector.tensor_copy(ident[:], ident_raw[:])
            w_raw = const_pool.tile([D, D], mybir.dt.float32)
            nc.sync.dma_start(out=w_raw[:], in_=w[:, :])
            w_sb = const_pool.tile([D, D], mybir.dt.float32)
            nc.vector.tensor_copy(w_sb[:], w_raw[:])

            half = TILES_PER_GROUP // 2
            for g in range(NGROUPS):
                if g == 0:
                    xt, xt3 = xt0, xt0_3
                else:
                    xt = io_pool.tile([P, TILES_PER_GROUP * TWO_D], mybir.dt.float32, tag="xt")
                    xt3 = xt[:].rearrange("p (t d) -> p t d", d=TWO_D)
                    nc.sync.dma_start(out=xt3, in_=xg[g])

                for b in range(TILES_PER_GROUP // BUNDLE):
                    pT = psT_pool.tile([P, BUNDLE * D], mybir.dt.float32)
                    for j in range(BUNDLE):
                        col = (b * BUNDLE + j) * TWO_D
                        nc.tensor.transpose(
                            pT[:, j * D:(j + 1) * D], xt[:, col:col + D], ident[:]
                        )
                    xTs = xT_pool.tile([P, BUNDLE * D], mybir.dt.float32)
                    nc.scalar.copy(out=xTs[:], in_=pT[:])
                    pH = psH_pool.tile([P, BUNDLE * D], mybir.dt.float32)
                    for j in range(BUNDLE):
                        nc.tensor.matmul(
                            pH[:, j * D:(j + 1) * D],
                            lhsT=xTs[:, j * D:(j + 1) * D],
                            rhs=w_sb[:],
                            start=True,
                            stop=True,
                        )
                    x2v = xt3[:, b * BUNDLE:(b + 1) * BUNDLE, D:TWO_D]
                    pHv = pH[:].rearrange("p (t d) -> p t d", d=D)
                    nc.vector.tensor_add(x2v, pHv, x2v)

                    # Flush each finished half of the group so the out
                    # DMA trails the adds instead of waiting for the
                    # whole group (shorter pipeline tail).
                    tiles_done = (b + 1) * BUNDLE
                    if tiles_done % half == 0:
                        h0 = tiles_done - half
                        nc.sync.dma_start(
                            out=og[g][:, h0:tiles_done],
                            in_=xt3[:, h0:tiles_done],
                        )

    _split_matmul_waits(nc)
    return nc


def _split_matmul_waits(nc: bass.Bass) -> None:
    """Several walrus ISA structs (Matmult's LDWEIGHTS uop, DVE
    TensorCopy, ...) encode only ONE sync-wait command; Tile sometimes
    emits 2+ ("Too many sync wait commands"). Hoist all but one wait
    onto standalone NoOps on the same queue right before the
    instruction — queue order makes this equivalent, and the hoisted
    waits are long-satisfied by then (they are stale WAW ticks)."""
    for blk in nc.cur_f.blocks:
        out = []
        for inst in blk.instructions:
            si = inst.sync_info
            if si is not None and si.on_wait and len(si.on_wait) > 1:
                waits = list(si.on_wait)
                for wait in waits[:-1]:
                    out.append(
                        mybir.InstNoOp(
                            name=nc.get_next_instruction_name(),
                            sync_info=mybir.SyncInfo(on_wait=[wait], on_update=[]),
                            engine=inst.engine,
                            bass_nofuse=True,
                        )
                    )
                inst.sync_info = mybir.SyncInfo(
                    on_wait=[waits[-1]], on_update=list(si.on_update or [])
                )
            out.append(inst)
        blk.instructions = out


def _get_nc() -> bass.Bass:
    if "nc" not in _CACHE:
        _CACHE["nc"] = _build_nc()
    return _CACHE["nc"]


def _in_maps(x: np.ndarray, weight: np.ndarray) -> list[dict[str, np.ndarray]]:
    x = np.ascontiguousarray(np.asarray(x, dtype=np.float32)).reshape(
        N_CORES, TOKENS, TWO_D
    )
    weight = np.ascontiguousarray(np.asarray(weight, dtype=np.float32))
    return [{"x": x[i], "weight": weight} for i in range(N_CORES)]


def kernel(x: np.ndarray, weight: np.ndarray) -> np.ndarray:
    nc = _get_nc()
    res = run_bass_kernel_spmd(nc, _in_maps(x, weight), core_ids=list(range(N_CORES)))
    out = np.stack([res.results[i]["out"] for i in range(N_CORES)], axis=0)
    return out.reshape(B, S, TWO_D)


# revision 33
# speedup vs baseline: 1.1872x; 1.1872x over previous
"""Trainium2 Bass kernel for nn_AMXReversibleLayer.

Reference computation (RevNet-style additive coupling):
    x1, x2 = split(x, 2, axis=-1)      # x: [B, S, 2D] f32, each [B, S, D]
    y = concat([x1, x2 + x1 @ W], -1)  # W: [D, D] f32

Strategy: pure data-parallel. x [8, 32768, 256] is sharded along batch —
one batch element (32768 tokens) per NeuronCore, W replicated. No
collectives. The kernel is memory-bound: per core 32 MB in + 32 MB out,
i.e. a floor of ~180 us at the ~358 GB/s per-core HBM limit. Measured
~176-200 us (run-to-run spread is HBM-stack contention between cores).

Per-core kernel (Tile framework):
  - Tokens live on SBUF partitions; the 256 features on the free axis.
    Each partition owns a CONTIGUOUS run of tokens, so per-partition
    DMA runs are tpg*1KB (64 KB) contiguous — minimal descriptor count
    at full line rate. Which 128 tokens form a compute tile is
    arbitrary, so compute is unaffected by this mapping.
  - TensorE needs the contraction dim (d) on partitions, so each
    128-token x1 tile is transposed on the PE array (matmul vs
    identity) into PSUM, copied back to SBUF (ScalarE), then matmul'd
    against W (out [tokens, e] in PSUM).
  - VectorE adds h in-place into the x2 columns of the input tile; the
    tile is flushed back to HBM in 1 MB slices that trail the adds.

Quirk handled by _split_matmul_waits: several walrus ISA structs
(Matmult's LDWEIGHTS uop most importantly) encode only ONE sync-wait
command, and Tile sometimes emits 2+ on one instruction ("Too many
sync wait commands" at codegen). The pass hoists extra waits onto
NoOps injected just before the instruction on the same queue.
"""

import numpy as np

import concourse.bass as bass
import concourse.mybir as mybir
from concourse.bass_utils import run_bass_kernel_spmd
from concourse.masks import make_identity
from concourse.tile import TileContext

N_CORES = 8
B, S, TWO_D = 8, 32768, 256
D = 128
P = 128

TOKENS = (B * S) // N_CORES          # tokens per core = 32768
TILES = TOKENS // P                  # 256 tiles of 128 tokens
TILES_PER_GROUP = 32                 # 32 tiles -> 4 MB in-DMA, 32 KB runs
NGROUPS = TILES // TILES_PER_GROUP   # 8
BUNDLE = 4                           # tiles per PSUM bank ([128, 512] f32)

_CACHE = {}


def _build_nc(
    tpg: int = 64,
    io_bufs: int = 3,
    bundle: int = BUNDLE,
    out_splits: int = 8,
    use_inline_ident: bool = False,
    small_first: bool = False,
) -> bass.Bass:
    ngroups = TILES // tpg
    nc = bass.Bass()
    x = nc.dram_tensor("x", [TOKENS, TWO_D], mybir.dt.float32, kind="ExternalInput")
    w = nc.dram_tensor("weight", [D, D], mybir.dt.float32, kind="ExternalInput")
    out = nc.dram_tensor("out", [TOKENS, TWO_D], mybir.dt.float32, kind="ExternalOutput")

    # [g, p, t, d] views: token = p*(ngroups*T) + g*T + t. Partition p
    # owns a CONTIGUOUS run of tokens, so each per-partition DMA run is
    # T*2D*4 bytes contiguous (vs 1 KB with interleaved mapping) —
    # far fewer descriptors at full line rate. Compute doesn't care
    # which 128 tokens form a tile.
    xg = x.rearrange("(p g t) d -> g p t d", p=P, g=ngroups)
    og = out.rearrange("(p g t) d -> g p t d", p=P, g=ngroups)

    with TileContext(nc) as tc:
        with (
            tc.tile_pool(name="const", bufs=1) as const_pool,
            tc.tile_pool(name="io", bufs=io_bufs) as io_pool,
            tc.tile_pool(name="xT", bufs=16 // bundle) as xT_pool,
            tc.tile_pool(name="psT", bufs=16 // bundle, space="PSUM") as psT_pool,
            tc.tile_pool(name="psH", bufs=16 // bundle, space="PSUM") as psH_pool,
        ):
            # Kick off the first input DMA before anything else so the
            # memory pipe starts streaming immediately. A small prefix
            # DMA first: descriptor generation for a full-size group
            # delays the first byte by several us, so let a 1 MB prefix
            # start the engines while the remainder's descriptors are
            # generated.
            xt0 = io_pool.tile([P, tpg * TWO_D], mybir.dt.float32, tag="xt")
            xt0_3 = xt0[:].rearrange("p (t d) -> p t d", d=TWO_D)
            pre = min(8, tpg) if small_first else tpg
            if pre < tpg:
                nc.sync.dma_start(out=xt0_3[:, 0:pre], in_=xg[0][:, 0:pre])
                nc.sync.dma_start(out=xt0_3[:, pre:], in_=xg[0][:, pre:])
            else:
                nc.sync.dma_start(out=xt0_3, in_=xg[0])

            if use_inline_ident:
                # Identity ships as a NEFF Const (loaded to HBM at model
                # load) — no gpsimd memset/affine_select at exec time.
                ident_dram = nc.inline_tensor(np.eye(P, dtype=np.float32), "identC")
                ident = const_pool.tile([P, P], mybir.dt.float32)
                nc.sync.dma_start(out=ident[:], in_=ident_dram[:, :])
                w_sb = const_pool.tile([D, D], mybir.dt.float32)
                nc.sync.dma_start(out=w_sb[:], in_=w[:, :])
            else:
                ident_raw = const_pool.tile([P, P], mybir.dt.float32)
                make_identity(nc, ident_raw[:])
                ident = const_pool.tile([P, P], mybir.dt.float32)
                nc.vector.tensor_copy(ident[:], ident_raw[:])
                w_raw = const_pool.tile([D, D], mybir.dt.float32)
                nc.sync.dma_start(out=w_raw[:], in_=w[:, :])
                w_sb = const_pool.tile([D, D], mybir.dt.float32)
                nc.vector.tensor_copy(w_sb[:], w_raw[:])

            split = tpg // out_splits
            for g in range(ngroups):
                if g == 0:
                    xt, xt3 = xt0, xt0_3
                else:
                    xt = io_pool.tile([P, tpg * TWO_D], mybir.dt.float32, tag="xt")
                    xt3 = xt[:].rearrange("p (t d) -> p t d", d=TWO_D)
                    nc.sync.dma_start(out=xt3, in_=xg[g])

                for b in range(tpg // bundle):
                    pT = psT_pool.tile([P, bundle * D], mybir.dt.float32)
                    for j in range(bundle):
                        col = (b * bundle + j) * TWO_D
                        nc.tensor.transpose(
                            pT[:, j * D:(j + 1) * D], xt[:, col:col + D], ident[:]
                        )
                    xTs = xT_pool.tile([P, bundle * D], mybir.dt.float32)
                    nc.scalar.copy(out=xTs[:], in_=pT[:])
                    pH = psH_pool.tile([P, bundle * D], mybir.dt.float32)
                    for j in range(bundle):
                        nc.tensor.matmul(
                            pH[:, j * D:(j + 1) * D],
                            lhsT=xTs[:, j * D:(j + 1) * D],
                            rhs=w_sb[:],
                            start=True,
                            stop=True,
                        )
                    x2v = xt3[:, b * bundle:(b + 1) * bundle, D:TWO_D]
                    pHv = pH[:].rearrange("p (t d) -> p t d", d=D)
                    nc.vector.tensor_add(x2v, pHv, x2v)

                    # Flush each finished slice of the group so the out
                    # DMA trails the adds instead of waiting for the
                    # whole group (shorter pipeline tail).
                    tiles_done = (b + 1) * bundle
                    if tiles_done % split == 0:
                        h0 = tiles_done - split
                        nc.sync.dma_start(
                            out=og[g][:, h0:tiles_done],
                            in_=xt3[:, h0:tiles_done],
                        )

    _split_matmul_waits(nc)
    return nc


def _split_matmul_waits(nc: bass.Bass) -> None:
    """Several walrus ISA structs (Matmult's LDWEIGHTS uop, DVE
    TensorCopy, ...) encode only ONE sync-wait command; Tile sometimes
    emits 2+ ("Too many sync wait commands"). Hoist all but one wait
    onto standalone NoOps on the same queue right before the
    instruction — queue order makes this equivalent, and the hoisted
    waits are long-satisfied by then (they are stale WAW ticks)."""
    for blk in nc.cur_f.blocks:
        out = []
        for inst in blk.instructions:
            si = inst.sync_info
            if si is not None and si.on_wait and len(si.on_wait) > 1:
                waits = list(si.on_wait)
                for wait in waits[:-1]:
                    out.append(
                        mybir.InstNoOp(
                            name=nc.get_next_instruction_name(),
                            sync_info=mybir.SyncInfo(on_wait=[wait], on_update=[]),
                            engine=inst.engine,
                            bass_nofuse=True,
                        )
                    )
                inst.sync_info = mybir.SyncInfo(
                    on_wait=[waits[-1]], on_update=list(si.on_update or [])
                )
            out.append(inst)
        blk.instructions = out


def _get_nc() -> bass.Bass:
    if "nc" not in _CACHE:
        _CACHE["nc"] = _build_nc()
    return _CACHE["nc"]


def _in_maps(x: np.ndarray, weight: np.ndarray) -> list[dict[str, np.ndarray]]:
    x = np.ascontiguousarray(np.asarray(x, dtype=np.float32)).reshape(
        N_CORES, TOKENS, TWO_D
    )
    weight = np.ascontiguousarray(np.asarray(weight, dtype=np.float32))
    return [{"x": x[i], "weight": weight} for i in range(N_CORES)]


def kernel(x: np.ndarray, weight: np.ndarray) -> np.ndarray:
    nc = _get_nc()
    res = run_bass_kernel_spmd(nc, _in_maps(x, weight), core_ids=list(range(N_CORES)))
    out = np.stack([res.results[i]["out"] for i in range(N_CORES)], axis=0)
    return out.reshape(B, S, TWO_D)
